# revision 7
# baseline (speedup 1.0000x reference)
"""Contrastive cosine-similarity MSE loss kernel for Trainium2 (8 cores).

Math (reference): scores_n = <a_n, b_n> / (||a_n|| * ||b_n||);
loss = mean((scores - labels)^2) over N=8192 rows, D=1024.

v2: the v1 kernel was compute-bound: all 24 row-stat reductions (8 blocks
x {dot, ||a||^2, ||b||^2}) ran as 1x-rate reduce ops on VectorE/ScalarE
(~17 us serialized) while DMA finished in ~12 us and TensorE sat idle.
Restructure:
  - ScalarE keeps 9 direct square-accumulate stats (na_0..7, nb_0).
  - VectorE computes the other 15 stats as plain fp16 tensor_tensor
    products (a*b, b*b) which run in the DVE's 2x_1P mode (~594 ns per
    [128,1024] block vs ~1136 ns for a 1x fused reduce).
  - TensorE folds each product [128,1024] -> PSUM [128,128] with 8
    identity-stationary accumulating matmuls (psum += chunk), then one
    segmented tensor_reduce per block finishes [128,s,128] -> [128,s].
  - PE warmup matmuls run during the DMA ramp so the fold matmuls hit
    the full 2.4 GHz p-state.
Embeddings are downcast to fp16 on the host (cosine is scale-invariant
to first order; measured end-to-end loss error ~e-7). All reductions
accumulate in fp32.

Sharding: data-parallel over rows; core c handles rows
[c*1024, (c+1)*1024). Tiles are [128 partitions x 2048] fp16 where
partition p holds rows (2p, 2p+1) of a 256-row block (4KB-contiguous
DRAM runs -> fat DMA packets). Block c = 2t+j (tile t, half j) has
row(p) = 256t + 2p + j; labels arrive in a matching [8, 128] layout and
are PE-transposed to [128, 8]. The final 128-partition partial SSE is
reduced to [1,1] with a ones-matmul; host sums the 8 per-core scalars.
"""

import numpy as np

import concourse.bacc as bacc
import concourse.bass as bass
import concourse.tile as tile
from concourse import mybir
from concourse.bass_utils import run_bass_kernel_spmd
from concourse.masks import make_identity
from concourse.vector_clock import ScopedClock


class _LeanTileContext(tile.TileContext):
    """TileContext with a minimal kernel epilogue.

    The stock epilogue is drain + all-engine butterfly + semaphore
    clear + second butterfly. For this single-shot kernel we only need
    the drain (all DMA queues complete, so the output is in DRAM before
    the NEFF retires); engines may retire their streams independently."""

    def _drain_and_barrier(self, tick_clock, wait_clock):
        drain_inst = self.nc.sync.drain()
        wait_clock.add_sem_waits(
            drain_inst.ins, ScopedClock({None: tick_clock.global_clock})
        )
        popped = self.nc._tile_sem_poison_stack.pop()
        assert popped is self._sem_poison


N, D = 8192, 1024
N_CORES = 8
ROWS = N // N_CORES  # rows per core
P = 128  # SBUF partitions
RPT = 2 * P  # rows per tile (2 per partition)
NTILES = ROWS // RPT  # 4
NBLK = 2 * NTILES  # 128-row blocks (tile t, half j -> c = 2t+j)
KCH = 8  # fold chunks per 1024-col product
PE_WARM = 24  # warmup matmuls to ramp the PE p-state

_cache = {}


def _build():
    nc = bacc.Bacc("TRN2", target_bir_lowering=False, debug=False)

    f32 = mybir.dt.float32
    f16 = mybir.dt.float16
    a = nc.dram_tensor("a", [ROWS, D], f16, kind="ExternalInput")
    b = nc.dram_tensor("b", [ROWS, D], f16, kind="ExternalInput")
    lab = nc.dram_tensor("lab_t", [NBLK, P], f32, kind="ExternalInput")
    out = nc.dram_tensor("out", [1, 1], f32, kind="ExternalOutput")

    with _LeanTileContext(nc) as tc:
        with (
            tc.tile_pool(name="io", bufs=NTILES) as io_pool,
            tc.tile_pool(name="prod", bufs=3) as prod_pool,
            tc.tile_pool(name="sq", bufs=2) as sq_pool,
            tc.tile_pool(name="fold", bufs=3, space="PSUM") as fold_pool,
            tc.tile_pool(name="psa", bufs=1, space="PSUM") as psa_pool,
            tc.tile_pool(name="stats", bufs=1) as st_pool,
        ):
            # --- upfront DMA: all 8 data tiles queued immediately ------
            ats, bts = [], []
            for t in range(NTILES):
                at = io_pool.tile([P, 2 * D], f16, tag="a")
                bt = io_pool.tile([P, 2 * D], f16, tag="b")
                a_src = bass.AP(
                    tensor=a, offset=t * RPT * D, ap=[[2 * D, P], [1, 2 * D]]
                )
                b_src = bass.AP(
                    tensor=b, offset=t * RPT * D, ap=[[2 * D, P], [1, 2 * D]]
                )
                # a first: ScalarE's first op (Square(a0)) needs only a.
                nc.sync.dma_start(out=at, in_=a_src)
                nc.sync.dma_start(out=bt, in_=b_src)
                ats.append(at)
                bts.append(bt)

            # Labels: one fat DMA into [NBLK, P], PE-transpose to [P, NBLK].
            lab_sb = st_pool.tile([NBLK, P], f32)
            nc.sync.dma_start(out=lab_sb, in_=lab[:, :])

            # --- constants -------------------------------------------
            na = st_pool.tile([P, NBLK], f32)
            # dot_c / nb_c interleaved: col 2c = dot_c, col 2c+1 = nb_c
            # (the per-block segmented reduce writes both in one op).
            stats_db = st_pool.tile([P, 2 * NBLK], f32)

            ones = st_pool.tile([P, 1], f32)
            nc.vector.memset(ones, 1.0)
            # Warm the Sqrt activation table while DMA ramps up.
            warm = st_pool.tile([P, 1], f32)
            nc.scalar.sqrt(warm, ones)

            id8 = st_pool.tile([NBLK, NBLK], f32)
            make_identity(nc, id8)
            labt = psa_pool.tile([P, NBLK], f32)
            nc.tensor.transpose(labt, lab_sb, id8)

            id128 = st_pool.tile([P, P], f16)
            make_identity(nc, id128)

            # PE warmup: identity@identity matmuls keep the PE busy
            # through its ~3us p-state ramp while the data DMA streams.
            wpsum = psa_pool.tile([P, P], f32, tag="warm")
            for w in range(PE_WARM):
                nc.tensor.matmul(wpsum, id128, id128[:, :])

            # --- main loop: 8 blocks of 128 rows ----------------------
            for c in range(NBLK):
                t, j = divmod(c, 2)
                asl = ats[t][:, j * D : (j + 1) * D]
                bsl = bts[t][:, j * D : (j + 1) * D]

                # ScalarE: na_c = sum a^2 (and nb_0 for block 0).
                sa = sq_pool.tile([P, D], f16, tag="sq")
                nc.scalar.activation(
                    out=sa,
                    in_=asl,
                    func=mybir.ActivationFunctionType.Square,
                    accum_out=na[:, c : c + 1],
                )
                if c == 0:
                    sb0 = sq_pool.tile([P, D], f16, tag="sq")
                    nc.scalar.activation(
                        out=sb0,
                        in_=bsl,
                        func=mybir.ActivationFunctionType.Square,
                        accum_out=stats_db[:, 1:2],
                    )

                # VectorE products (2x_1P fp16): halves of one scratch
                # tile so the PE fold reads both with a single 3D AP.
                nslot = 1 if c == 0 else 2
                pt = prod_pool.tile([P, nslot * D], f16, tag="p")
                nc.vector.tensor_mul(pt[:, 0:D], asl, bsl)
                if c > 0:
                    nc.vector.tensor_mul(pt[:, D : 2 * D], bsl, bsl)

                # TensorE: fold [P, nslot*1024] -> PSUM [P, nslot, 128]
                # by accumulating the 8 column-chunks of each half.
                fps = fold_pool.tile([P, nslot, P], f32)
                pt4 = pt[:, :].rearrange(
                    "p (s k c) -> p s k c", s=nslot, k=KCH, c=P
                )
                for k in range(KCH):
                    nc.tensor.matmul(
                        fps,
                        id128,
                        pt4[:, :, k, :],
                        start=(k == 0),
                        stop=(k == KCH - 1),
                    )

                # VectorE: segmented reduce -> (dot_c, nb_c).
                nc.vector.tensor_reduce(
                    out=stats_db[:, 2 * c : 2 * c + nslot],
                    in_=fps,
                    axis=mybir.AxisListType.X,
                    op=mybir.AluOpType.add,
                )

            # --- tail on [P, NBLK] stats (tiny, fp32) -----------------
            dots = stats_db[:, 0 : 2 * NBLK : 2]
            nb = stats_db[:, 1 : 2 * NBLK : 2]
            prod = st_pool.tile([P, NBLK], f32)
            nc.vector.tensor_mul(prod, na, nb)
            nc.scalar.sqrt(prod, prod)
            rs = st_pool.tile([P, NBLK], f32)
            nc.vector.reciprocal(rs, prod)
            score = st_pool.tile([P, NBLK], f32)
            nc.vector.tensor_mul(score, dots, rs)
            diff = st_pool.tile([P, NBLK], f32)
            nc.vector.tensor_sub(diff, score, labt)
            sqd = st_pool.tile([P, NBLK], f32)
            partial = st_pool.tile([P, 1], f32)
            nc.vector.scalar_tensor_tensor(
                out=sqd,
                in0=diff,
                scalar=1.0,
                in1=diff,
                op0=mybir.AluOpType.mult,
                op1=mybir.AluOpType.mult,
                accum_out=partial,
            )
            # Reduce 128 partitions -> [1,1] so the output DMA is one
            # descriptor instead of 128.
            total_ps = psa_pool.tile([1, 1], f32)
            nc.tensor.matmul(total_ps, partial, ones)
            res_sb = st_pool.tile([1, 1], f32)
            nc.scalar.copy(res_sb, total_ps)
            nc.sync.dma_start(out=out[:, :], in_=res_sb)

    nc.compile()
    return nc


def _label_perm(lab_core):
    """[ROWS] -> [NBLK, P] so that PE-transpose yields labt[p, c] =
    labels[256*(c//2) + 2p + (c%2)], matching the stats layout."""
    return np.ascontiguousarray(
        lab_core.reshape(NTILES, P, 2).transpose(0, 2, 1).reshape(NBLK, P)
    )


def kernel(issues_1_geb, issues_2_geb, labels):
    if "nc" not in _cache:
        _cache["nc"] = _build()
    nc = _cache["nc"]

    a16 = np.ascontiguousarray(issues_1_geb, dtype=np.float16)
    b16 = np.ascontiguousarray(issues_2_geb, dtype=np.float16)
    lab = np.ascontiguousarray(labels, dtype=np.float32)

    in_maps = []
    for c in range(N_CORES):
        sl = slice(c * ROWS, (c + 1) * ROWS)
        in_maps.append(
            {
                "a": np.ascontiguousarray(a16[sl]),
                "b": np.ascontiguousarray(b16[sl]),
                "lab_t": _label_perm(lab[sl]),
            }
        )

    res = run_bass_kernel_spmd(nc, in_maps, core_ids=list(range(N_CORES)))
    total = np.float64(0.0)
    for r in res.results:
        total += np.float64(r["out"].sum(dtype=np.float64))
    return np.array(total / N, dtype=np.float32)
